# revision 1
# baseline (speedup 1.0000x reference)
"""Causal self-attention (B=4, S=2048, D=1024, single head) on 8 trn2 cores.

Sharding: data-parallel over batch (4 batches x 2 cores). The two cores of a
batch split the 8 query tiles of 256 rows by parity (core even: tiles
{0,2,4,6}, odd: {1,3,5,7}) so every core runs the *same* block schedule
(uniform SPMD program); causality and tile position enter only through
per-core input data (query-row gather + additive mask tiles).

Per-core kernel: project K^T / V / Q^T with fp32r matmuls (X and W transposed
on the tensor engine against an identity), stage V through a DRAM scratch to
fit SBUF, then blocked attention: scoresT[k,q] blocks accumulate in PSUM,
additive causal mask (DVE), exp via ScalarE (scale=1/sqrt(D)) straight into
fp32r SBUF tiles that feed the AV matmuls; row-sums ride along as an extra
N=2 matmul against a ones vector sharing the AV lhsT; normalize fuses into
the PSUM->SBUF eviction as a per-partition scalar multiply.
"""

import os
import numpy as np
from contextlib import ExitStack

import concourse.bass as bass
import concourse.tile as tile
import concourse.mybir as mybir
from concourse import bacc
from concourse.bass_utils import run_bass_kernel_spmd
from concourse.masks import make_identity

F32 = mybir.dt.float32
F32R = mybir.dt.float32r
AFT = mybir.ActivationFunctionType

B, S, D = 4, 2048, 1024
P = 128
QTILE = 256          # queries per attention tile (4 tiles/core)
NT = 4               # attention tiles per core
SB = 256             # rows per projection s-block
NSB = S // SB        # 8
DC = D // P          # 8 contraction chunks
SCALE = 1.0 / np.sqrt(D)
MASK_NEG = -1.0e9

N_KC = [4 * (t + 1) for t in range(NT)]          # kc blocks per tile: 4,8,12,16
NBLK = sum(N_KC)                                  # 40
BLK0 = [sum(N_KC[:t]) for t in range(NT)]         # block offsets per tile

_NC_CACHE = None
_PHASES = os.environ.get("KPHASES", "BCD")  # which phases to emit (sim attribution)


def _build():
    nc = bacc.Bacc("TRN2", target_bir_lowering=False, debug=False, num_devices=8)
    xq = nc.dram_tensor("Xq", [NT * QTILE, D], F32, kind="ExternalInput").ap()
    xkv = nc.dram_tensor("Xkv", [S, D], F32, kind="ExternalInput").ap()
    wq = nc.dram_tensor("Wq", [D, D], F32, kind="ExternalInput").ap()
    wk = nc.dram_tensor("Wk", [D, D], F32, kind="ExternalInput").ap()
    wv = nc.dram_tensor("Wv", [D, D], F32, kind="ExternalInput").ap()
    msk = nc.dram_tensor("Mask", [NT, 4, P, QTILE], F32, kind="ExternalInput").ap()
    out = nc.dram_tensor("O", [NT * QTILE, D], F32, kind="ExternalOutput").ap()

    HK = S // (2 * P)   # 8 kc chunks per K/V half

    with tile.TileContext(nc) as tc, ExitStack() as ctx:
        big = ctx.enter_context(tc.tile_pool(name="big", bufs=1))
        dram = ctx.enter_context(tc.tile_pool(name="dram", bufs=1, space="DRAM"))

        ident = big.tile([P, P], F32)
        make_identity(nc, ident[:])
        ones_f = big.tile([P, 2], F32)
        nc.vector.memset(ones_f[:], 1.0)
        ones2 = big.tile([P, 2], F32R)
        nc.vector.tensor_copy(ones2[:], ones_f[:])

        # resident low halves (k < 1024); high halves staged via DRAM
        KTlo = big.tile([P, DC, S // 2], F32R)     # [e-part, ec, k]
        Vlo = big.tile([P, HK, D], F32R)           # [k-part, kc, e]
        KTdram = dram.tile([DC, P, S // 2], F32R)
        Vdram = dram.tile([S // 2, D], F32R)

        tb_ctr = [0]

        def transpose_block(psum, src_ap, dst_ap):
            """PE-transpose a [128,128] f32 block; evict psum->sbuf rounds to f32r.
            Evictions go 3:1 DVE:ACT."""
            pt = psum.tile([P, P], F32, tag="tp")
            nc.tensor.transpose(pt[:], src_ap, ident[:])
            tb_ctr[0] += 1
            if tb_ctr[0] % 4 == 0:
                nc.scalar.copy(dst_ap, pt[:])
            else:
                nc.vector.tensor_copy(dst_ap, pt[:])

        def load_wT(wpool, spool, psum, wap, name):
            """Load W [e,d] natural, PE-transpose into W^T [d-part, dc, e] fp32r."""
            wt = wpool.tile([P, DC, D], F32R, tag=f"wt_{name}")
            for ec in range(DC):
                wn = spool.tile([P, D], F32, tag="wnat")
                nc.sync.dma_start(wn[:], wap[ec * P:(ec + 1) * P, :])
                for dc in range(DC):
                    transpose_block(psum, wn[:, dc * P:(dc + 1) * P],
                                    wt[:, dc, ec * P:(ec + 1) * P])
            return wt

        # ---------------- Phase B: K^T and V ----------------
        if "B" in _PHASES:
         with tc.tile_pool(name="projB_w", bufs=1) as pbw, \
             tc.tile_pool(name="projB", bufs=2) as pb, \
             tc.tile_pool(name="projB_x", bufs=1) as pbx, \
             tc.tile_pool(name="projB_ps", bufs=2, space="PSUM") as pbps:
            # hi s-blocks (DRAM-staged) first so phase B's tail has no pending
            # stores; lo s-blocks (SBUF-resident evictions) last. Prefetch +
            # transpose the first s-block of X before the 8MB of W loads.
            SB_ORDER = [4, 5, 6, 7, 0, 1, 2, 3]
            xt0 = pbx.tile([P, DC, SB], F32R, tag="xt")
            for i in range(SB // P):
                xn = pb.tile([P, D], F32, tag="xn")
                r = SB_ORDER[0] * (SB // P) + i
                nc.sync.dma_start(xn[:], xkv[r * P:(r + 1) * P, :])
                for dc in range(DC):
                    transpose_block(pbps, xn[:, dc * P:(dc + 1) * P],
                                    xt0[:, dc, i * P:(i + 1) * P])
            wkt = load_wT(pbw, pb, pbps, wk, "k")
            wvt = load_wT(pbw, pb, pbps, wv, "v")
            for si, sb in enumerate(SB_ORDER):
                lo = sb < NSB // 2
                if si == 0:
                    xt = xt0
                else:
                    xt = pbx.tile([P, DC, SB], F32R, tag="xt")
                    for i in range(SB // P):
                        xn = pb.tile([P, D], F32, tag="xn")
                        nc.sync.dma_start(xn[:], xkv[(sb * (SB // P) + i) * P:(sb * (SB // P) + i + 1) * P, :])
                        for dc in range(DC):
                            transpose_block(pbps, xn[:, dc * P:(dc + 1) * P],
                                            xt[:, dc, i * P:(i + 1) * P])
                # K^T columns for this s-block
                if lo:
                    for ec in range(DC):
                        pk = pbps.tile([P, SB], F32, tag="pk")
                        for dc in range(DC):
                            nc.tensor.matmul(pk[:], wkt[:, dc, ec * P:(ec + 1) * P],
                                             xt[:, dc, :], start=(dc == 0), stop=(dc == DC - 1))
                        nc.scalar.copy(KTlo[:, ec, sb * SB:(sb + 1) * SB], pk[:])
                else:
                    kst = pb.tile([P, DC, SB], F32R, tag="kst")
                    for ec in range(DC):
                        pk = pbps.tile([P, SB], F32, tag="pk")
                        for dc in range(DC):
                            nc.tensor.matmul(pk[:], wkt[:, dc, ec * P:(ec + 1) * P],
                                             xt[:, dc, :], start=(dc == 0), stop=(dc == DC - 1))
                        nc.scalar.copy(kst[:, ec, :], pk[:])
                    nc.sync.dma_start(
                        KTdram[:, :, (sb - NSB // 2) * SB:(sb - NSB // 2 + 1) * SB].rearrange("ec p k -> p ec k"),
                        kst[:])
                # V rows for this s-block
                if lo:
                    for i in range(SB // P):
                        for e2 in range(2):
                            pv = pbps.tile([P, 512], F32, tag="pv")
                            for dc in range(DC):
                                nc.tensor.matmul(pv[:], xt[:, dc, i * P:(i + 1) * P],
                                                 wvt[:, dc, e2 * 512:(e2 + 1) * 512],
                                                 start=(dc == 0), stop=(dc == DC - 1))
                            nc.scalar.copy(Vlo[:, sb * (SB // P) + i, e2 * 512:(e2 + 1) * 512], pv[:])
                else:
                    vst = pb.tile([P, SB // P, D], F32R, tag="vst")
                    for i in range(SB // P):
                        for e2 in range(2):
                            pv = pbps.tile([P, 512], F32, tag="pv")
                            for dc in range(DC):
                                nc.tensor.matmul(pv[:], xt[:, dc, i * P:(i + 1) * P],
                                                 wvt[:, dc, e2 * 512:(e2 + 1) * 512],
                                                 start=(dc == 0), stop=(dc == DC - 1))
                            nc.scalar.copy(vst[:, i, e2 * 512:(e2 + 1) * 512], pv[:])
                    nc.sync.dma_start(
                        Vdram[(sb - NSB // 2) * SB:(sb - NSB // 2 + 1) * SB, :].rearrange("(i p) e -> p i e", p=P),
                        vst[:])

        # ---------------- Phases C+D persistent ----------------
        persist2 = ctx.enter_context(tc.tile_pool(name="persist2", bufs=1))
        QT = persist2.tile([P, DC, NT * QTILE], F32R)  # Q^T [e-part, ec, q]

        # ---------------- Phase C: Q^T ----------------
        if "C" in _PHASES:
         with tc.tile_pool(name="projC_w", bufs=1) as pcw, \
             tc.tile_pool(name="projC", bufs=4) as pc, \
             tc.tile_pool(name="projC_ps", bufs=2, space="PSUM") as pcps:
            xtq = pcw.tile([P, DC, NT * QTILE], F32R, tag="xtq")
            for i in range(NT * QTILE // P):
                xn = pc.tile([P, D], F32, tag="xn")
                nc.sync.dma_start(xn[:], xq[i * P:(i + 1) * P, :])
                for dc in range(DC):
                    transpose_block(pcps, xn[:, dc * P:(dc + 1) * P],
                                    xtq[:, dc, i * P:(i + 1) * P])
            wqt = load_wT(pcw, pc, pcps, wq, "q")
            for ec in range(DC):
                for qc in range(NT * QTILE // 512):
                    pq = pcps.tile([P, 512], F32, tag="pq")
                    for dc in range(DC):
                        nc.tensor.matmul(pq[:], wqt[:, dc, ec * P:(ec + 1) * P],
                                         xtq[:, dc, qc * 512:(qc + 1) * 512],
                                         start=(dc == 0), stop=(dc == DC - 1))
                    nc.scalar.copy(QT[:, ec, qc * 512:(qc + 1) * 512], pq[:])

        # ---------------- Phase D: attention ----------------
        if "D" in _PHASES:
         with tc.tile_pool(name="attn", bufs=2) as pa, \
             tc.tile_pool(name="attn_e", bufs=1) as pe_pool, \
             tc.tile_pool(name="attn_m", bufs=2) as pm, \
             tc.tile_pool(name="attn_o", bufs=1) as po, \
             tc.tile_pool(name="hi", bufs=1) as phi, \
             tc.tile_pool(name="attn_s", bufs=3, space="PSUM") as psS, \
             tc.tile_pool(name="attn_u", bufs=2, space="PSUM") as psU, \
             tc.tile_pool(name="attn_r", bufs=1, space="PSUM") as psR:
            KThi = phi.tile([P, DC, S // 2], F32R)
            Vhi = phi.tile([P, HK, D], F32R)
            # tile-0 mask first so it isn't queued behind the 8MB hi loads
            mtiles = []
            m0 = pm.tile([P, 4, QTILE], F32, tag="mtile")
            nc.gpsimd.dma_start(m0[:], msk[0].rearrange("b p j -> p b j"))
            mtiles.append(m0)
            # split hi reloads: tile 2 needs only kc 8..11, tile 3 the rest
            nc.sync.dma_start(KThi[:, :, 0:512],
                              KTdram[:, :, 0:512].rearrange("ec p k -> p ec k"))
            nc.sync.dma_start(Vhi[:, 0:4, :],
                              Vdram[0:512, :].rearrange("(kc p) e -> p kc e", p=P))
            nc.sync.dma_start(KThi[:, :, 512:1024],
                              KTdram[:, :, 512:1024].rearrange("ec p k -> p ec k"))
            nc.sync.dma_start(Vhi[:, 4:8, :],
                              Vdram[512:1024, :].rearrange("(kc p) e -> p kc e", p=P))

            def KTat(ec, kc):
                if kc < HK:
                    return KTlo[:, ec, kc * P:(kc + 1) * P]
                return KThi[:, ec, (kc - HK) * P:(kc - HK + 1) * P]

            def Vat(kc, esl):
                if kc < HK:
                    return Vlo[:, kc, esl]
                return Vhi[:, kc - HK, esl]

            for t in range(NT):
                n = N_KC[t]
                mtile = mtiles[t]
                if t + 1 < NT:  # prefetch next tile's mask blocks
                    mnext = pm.tile([P, 4, QTILE], F32, tag="mtile")
                    nc.gpsimd.dma_start(mnext[:], msk[t + 1].rearrange("b p j -> p b j"))
                    mtiles.append(mnext)
                expS = pe_pool.tile([P, 16, QTILE], F32R, tag="expS")
                for kc in range(n):
                    pS = psS.tile([P, QTILE], F32, tag="pS")
                    for ec in range(DC):
                        nc.tensor.matmul(pS[:], KTat(ec, kc),
                                         QT[:, ec, t * QTILE:(t + 1) * QTILE],
                                         start=(ec == 0), stop=(ec == DC - 1))
                    if kc >= n - 4:  # only the 4 diagonal-edge blocks carry a mask
                        nc.vector.tensor_add(pS[:], pS[:], mtile[:, kc - (n - 4), :])
                    nc.scalar.activation(expS[:, kc, :], pS[:], AFT.Exp, scale=SCALE)
                for qc in range(QTILE // P):
                    pU0 = psU.tile([P, 512], F32, tag="pU0")
                    pU1 = psU.tile([P, 512], F32, tag="pU1")
                    pR = psR.tile([P, 2], F32, tag="pR")
                    for kc in range(n):
                        lhs = expS[:, kc, qc * P:(qc + 1) * P]
                        st, sp = (kc == 0), (kc == n - 1)
                        nc.tensor.matmul(pU0[:], lhs, Vat(kc, slice(0, 512)), start=st, stop=sp)
                        nc.tensor.matmul(pU1[:], lhs, Vat(kc, slice(512, 1024)), start=st, stop=sp)
                        nc.tensor.matmul(pR[:], lhs, ones2[:], start=st, stop=sp)
                    rsb = pa.tile([P, 1], F32, tag="rsb")
                    recip = pa.tile([P, 1], F32, tag="recip")
                    nc.vector.tensor_copy(rsb[:], pR[:, 0:1])
                    nc.vector.reciprocal(recip[:], rsb[:])
                    ot = po.tile([P, D], F32, tag="ot")
                    nc.vector.tensor_scalar_mul(ot[:, 0:512], pU0[:], recip[:])
                    nc.vector.tensor_scalar_mul(ot[:, 512:1024], pU1[:], recip[:])
                    nc.sync.dma_start(out[(t * QTILE + qc * P):(t * QTILE + (qc + 1) * P), :], ot[:])

    nc.compile()
    return nc


def _get_nc():
    global _NC_CACHE
    if _NC_CACHE is None:
        _NC_CACHE = _build()
    return _NC_CACHE


def _make_masks(parity: int) -> np.ndarray:
    """Masks for the last 4 kc blocks of each tile (earlier blocks are fully
    visible for both parities)."""
    m = np.empty((NT, 4, P, QTILE), dtype=np.float32)
    j = np.arange(QTILE)[None, :]
    p = np.arange(P)[:, None]
    for t in range(NT):
        g = 2 * t + parity
        n = N_KC[t]
        for s in range(4):
            kc = n - 4 + s
            qglob = g * QTILE + j
            kglob = kc * P + p
            m[t, s] = np.where(qglob >= kglob, 0.0, MASK_NEG)
    return m


def kernel(X, W_q, W_k, W_v):
    X = np.asarray(X, dtype=np.float32)
    W_q = np.asarray(W_q, dtype=np.float32)
    W_k = np.asarray(W_k, dtype=np.float32)
    W_v = np.asarray(W_v, dtype=np.float32)

    masks = [_make_masks(par) for par in range(2)]
    in_maps = []
    for c in range(8):
        b, par = c // 2, c % 2
        rows = np.concatenate([X[b, (2 * t + par) * QTILE:(2 * t + par + 1) * QTILE, :]
                               for t in range(NT)], axis=0)
        in_maps.append({
            "Xq": np.ascontiguousarray(rows),
            "Xkv": np.ascontiguousarray(X[b]),
            "Wq": W_q, "Wk": W_k, "Wv": W_v,
            "Mask": masks[par],
        })

    global _last_in_maps
    _last_in_maps = in_maps
    nc = _get_nc()
    res = run_bass_kernel_spmd(nc, in_maps, core_ids=list(range(8)))

    out = np.empty((B, S, D), dtype=np.float32)
    for c in range(8):
        b, par = c // 2, c % 2
        oc = res.results[c]["O"]
        for t in range(NT):
            g = 2 * t + par
            out[b, g * QTILE:(g + 1) * QTILE, :] = oc[t * QTILE:(t + 1) * QTILE, :]
    return out



# revision 2
# speedup vs baseline: 1.1180x; 1.1180x over previous
"""Causal self-attention (B=4, S=2048, D=1024, single head) on 8 trn2 cores.

Sharding: data-parallel over batch (4 batches x 2 cores). The two cores of a
batch split the 8 query strips of 256 rows by parity (core even: strips
{0,2,4,6}, odd: {1,3,5,7}) so every core runs the *same* block schedule
(uniform SPMD program); causality and strip position enter only through
per-core input data (query-column gather + additive mask tiles).

v2 vs the fp32r baseline:
- All matmul operands are bf16 (same 1 cycle/row PE rate as fp32r at N>=256,
  half the SBUF/DMA footprint; fp32 PSUM accumulation keeps rel err ~3e-3).
- X^T and W^T are pre-transposed on the host, eliminating all on-device PE
  transposes (~74k PE cycles) and their PSUM evictions.
- K^T, V, Q^T live entirely in SBUF (no DRAM staging round trip).
Per-core PE floor: proj 328k rows + attention 164k rows ~= 205us @2.4GHz.
"""

import numpy as np
from contextlib import ExitStack

import ml_dtypes

import concourse.bass as bass
import concourse.tile as tile
import concourse.mybir as mybir
from concourse import bacc
from concourse.bass_utils import run_bass_kernel_spmd

F32 = mybir.dt.float32
BF16 = mybir.dt.bfloat16
AFT = mybir.ActivationFunctionType
BF = ml_dtypes.bfloat16

B, S, D = 4, 2048, 1024
P = 128
QTILE = 256          # queries per attention tile (4 tiles/core)
NT = 4               # attention tiles per core
DC = D // P          # 8 contraction chunks
HK = S // P          # 16 key chunks of 128
SCALE = 1.0 / np.sqrt(D)
MASK_NEG = -1.0e9

N_KC = [4 * (t + 1) for t in range(NT)]          # kc blocks per tile: 4,8,12,16

_NC_CACHE = None


def _build():
    nc = bacc.Bacc("TRN2", target_bir_lowering=False, debug=False, num_devices=8)
    xt = nc.dram_tensor("XT", [D, S], BF16, kind="ExternalInput").ap()
    xqt = nc.dram_tensor("XqT", [D, NT * QTILE], BF16, kind="ExternalInput").ap()
    wqt = nc.dram_tensor("WqT", [D, D], BF16, kind="ExternalInput").ap()
    wkt = nc.dram_tensor("WkT", [D, D], BF16, kind="ExternalInput").ap()
    wvt = nc.dram_tensor("WvT", [D, D], BF16, kind="ExternalInput").ap()
    msk = nc.dram_tensor("Mask", [NT, 4, P, QTILE], F32, kind="ExternalInput").ap()
    out = nc.dram_tensor("O", [NT * QTILE, D], F32, kind="ExternalOutput").ap()

    with tile.TileContext(nc) as tc, ExitStack() as ctx:
        persist = ctx.enter_context(tc.tile_pool(name="persist", bufs=1))

        ones_f = persist.tile([P, 2], F32)
        nc.vector.memset(ones_f[:], 1.0)
        ones2 = persist.tile([P, 2], BF16)
        nc.vector.tensor_copy(ones2[:], ones_f[:])
        # preload the Exp activation table during the projection phase so the
        # first attention exp doesn't pay the ~1.3us table-load latency
        warm = persist.tile([P, 2], F32)
        nc.scalar.activation(warm[:], ones_f[:], AFT.Exp, scale=1.0)

        KT = persist.tile([P, DC, S], BF16)        # [e-part, ec, k]
        V = persist.tile([P, HK, D], BF16)         # [k-part, kc, e]
        QT = persist.tile([P, DC, NT * QTILE], BF16)  # [e-part, ec, q]

        ev_ctr = [0]

        def evict(dst_ap, src_ap):
            """PSUM->SBUF bf16 eviction, alternating DVE/ACT."""
            ev_ctr[0] += 1
            if ev_ctr[0] % 2 == 0:
                nc.scalar.copy(dst_ap, src_ap)
            else:
                nc.vector.tensor_copy(dst_ap, src_ap)

        # ---------------- projections ----------------
        with tc.tile_pool(name="proj_in", bufs=1) as pin, \
             tc.tile_pool(name="proj_ps", bufs=2, space="PSUM") as pps:
            XTs = pin.tile([P, DC, S], BF16, tag="xts")
            WkTs = pin.tile([P, DC, D], BF16, tag="wkts")
            WvTs = pin.tile([P, DC, D], BF16, tag="wvts")
            WqTs = pin.tile([P, DC, D], BF16, tag="wqts")
            XqTs = pin.tile([P, DC, NT * QTILE], BF16, tag="xqts")
            # interleave so the first K-proj matmuls can start after ~2 chunks
            for dc in range(DC):
                nc.sync.dma_start(XTs[:, dc, :], xt[dc * P:(dc + 1) * P, :])
                nc.sync.dma_start(WkTs[:, dc, :], wkt[dc * P:(dc + 1) * P, :])
            for dc in range(DC):
                nc.sync.dma_start(WvTs[:, dc, :], wvt[dc * P:(dc + 1) * P, :])
            for dc in range(DC):
                nc.sync.dma_start(WqTs[:, dc, :], wqt[dc * P:(dc + 1) * P, :])
                nc.sync.dma_start(XqTs[:, dc, :], xqt[dc * P:(dc + 1) * P, :])

            # K^T[e, k] = sum_d WkT[d, e] * XT[d, k]
            for ec in range(DC):
                for kch in range(S // 512):
                    pk = pps.tile([P, 512], F32, tag="pk")
                    for dc in range(DC):
                        nc.tensor.matmul(pk[:], WkTs[:, dc, ec * P:(ec + 1) * P],
                                         XTs[:, dc, kch * 512:(kch + 1) * 512],
                                         start=(dc == 0), stop=(dc == DC - 1))
                    evict(KT[:, ec, kch * 512:(kch + 1) * 512], pk[:])
            # V[k, e] = sum_d XT[d, k] * WvT[d, e]
            for kb in range(HK):
                for eh in range(2):
                    pv = pps.tile([P, 512], F32, tag="pv")
                    for dc in range(DC):
                        nc.tensor.matmul(pv[:], XTs[:, dc, kb * P:(kb + 1) * P],
                                         WvTs[:, dc, eh * 512:(eh + 1) * 512],
                                         start=(dc == 0), stop=(dc == DC - 1))
                    evict(V[:, kb, eh * 512:(eh + 1) * 512], pv[:])
            # Q^T[e, q] = sum_d WqT[d, e] * XqT[d, q]
            for ec in range(DC):
                for qh in range(2):
                    pq = pps.tile([P, 512], F32, tag="pq")
                    for dc in range(DC):
                        nc.tensor.matmul(pq[:], WqTs[:, dc, ec * P:(ec + 1) * P],
                                         XqTs[:, dc, qh * 512:(qh + 1) * 512],
                                         start=(dc == 0), stop=(dc == DC - 1))
                    evict(QT[:, ec, qh * 512:(qh + 1) * 512], pq[:])

        # ---------------- attention ----------------
        with tc.tile_pool(name="attn", bufs=2) as pa, \
             tc.tile_pool(name="attn_e", bufs=2) as pe_pool, \
             tc.tile_pool(name="attn_m", bufs=2) as pm, \
             tc.tile_pool(name="attn_o", bufs=2) as po, \
             tc.tile_pool(name="attn_s", bufs=3, space="PSUM") as psS, \
             tc.tile_pool(name="attn_u", bufs=2, space="PSUM") as psU, \
             tc.tile_pool(name="attn_r", bufs=1, space="PSUM") as psR:
            mtiles = []
            m0 = pm.tile([P, 4, QTILE], F32, tag="mtile")
            nc.gpsimd.dma_start(m0[:], msk[0].rearrange("b p j -> p b j"))
            mtiles.append(m0)

            for t in range(NT):
                n = N_KC[t]
                mtile = mtiles[t]
                if t + 1 < NT:  # prefetch next tile's mask blocks
                    mnext = pm.tile([P, 4, QTILE], F32, tag="mtile")
                    nc.gpsimd.dma_start(mnext[:], msk[t + 1].rearrange("b p j -> p b j"))
                    mtiles.append(mnext)
                expS = pe_pool.tile([P, HK, QTILE], BF16, tag="expS")
                for kc in range(n):
                    pS = psS.tile([P, QTILE], F32, tag="pS")
                    for ec in range(DC):
                        nc.tensor.matmul(pS[:], KT[:, ec, kc * P:(kc + 1) * P],
                                         QT[:, ec, t * QTILE:(t + 1) * QTILE],
                                         start=(ec == 0), stop=(ec == DC - 1))
                    if kc >= n - 4:  # only the 4 diagonal-edge blocks carry a mask
                        nc.vector.tensor_add(pS[:], pS[:], mtile[:, kc - (n - 4), :])
                    nc.scalar.activation(expS[:, kc, :], pS[:], AFT.Exp, scale=SCALE)
                for qc in range(QTILE // P):
                    pU0 = psU.tile([P, 512], F32, tag="pU0")
                    pU1 = psU.tile([P, 512], F32, tag="pU1")
                    pR = psR.tile([P, 2], F32, tag="pR")
                    for kc in range(n):
                        lhs = expS[:, kc, qc * P:(qc + 1) * P]
                        st, sp = (kc == 0), (kc == n - 1)
                        nc.tensor.matmul(pU0[:], lhs, V[:, kc, 0:512], start=st, stop=sp)
                        nc.tensor.matmul(pU1[:], lhs, V[:, kc, 512:1024], start=st, stop=sp)
                        nc.tensor.matmul(pR[:], lhs, ones2[:], start=st, stop=sp)
                    rsb = pa.tile([P, 1], F32, tag="rsb")
                    recip = pa.tile([P, 1], F32, tag="recip")
                    nc.vector.tensor_copy(rsb[:], pR[:, 0:1])
                    nc.vector.reciprocal(recip[:], rsb[:])
                    ot = po.tile([P, D], F32, tag="ot")
                    nc.vector.tensor_scalar_mul(ot[:, 0:512], pU0[:], recip[:])
                    nc.vector.tensor_scalar_mul(ot[:, 512:1024], pU1[:], recip[:])
                    nc.sync.dma_start(out[(t * QTILE + qc * P):(t * QTILE + (qc + 1) * P), :], ot[:])

    nc.compile()
    return nc


def _get_nc():
    global _NC_CACHE
    if _NC_CACHE is None:
        _NC_CACHE = _build()
    return _NC_CACHE


def _make_masks(parity: int) -> np.ndarray:
    """Masks for the last 4 kc blocks of each tile (earlier blocks are fully
    visible for both parities)."""
    m = np.empty((NT, 4, P, QTILE), dtype=np.float32)
    j = np.arange(QTILE)[None, :]
    p = np.arange(P)[:, None]
    for t in range(NT):
        g = 2 * t + parity
        n = N_KC[t]
        for s in range(4):
            kc = n - 4 + s
            qglob = g * QTILE + j
            kglob = kc * P + p
            m[t, s] = np.where(qglob >= kglob, 0.0, MASK_NEG)
    return m


def kernel(X, W_q, W_k, W_v):
    X = np.asarray(X, dtype=np.float32)
    WqT = np.ascontiguousarray(np.asarray(W_q, dtype=np.float32).T).astype(BF)
    WkT = np.ascontiguousarray(np.asarray(W_k, dtype=np.float32).T).astype(BF)
    WvT = np.ascontiguousarray(np.asarray(W_v, dtype=np.float32).T).astype(BF)

    XT_all = [np.ascontiguousarray(X[b].T).astype(BF) for b in range(B)]
    masks = [_make_masks(par) for par in range(2)]
    in_maps = []
    for c in range(8):
        b, par = c // 2, c % 2
        xqt = np.concatenate(
            [XT_all[b][:, (2 * t + par) * QTILE:(2 * t + par + 1) * QTILE]
             for t in range(NT)], axis=1)
        in_maps.append({
            "XT": XT_all[b],
            "XqT": np.ascontiguousarray(xqt),
            "WqT": WqT, "WkT": WkT, "WvT": WvT,
            "Mask": masks[par],
        })

    global _last_in_maps
    _last_in_maps = in_maps
    nc = _get_nc()
    res = run_bass_kernel_spmd(nc, in_maps, core_ids=list(range(8)))

    out = np.empty((B, S, D), dtype=np.float32)
    for c in range(8):
        b, par = c // 2, c % 2
        oc = res.results[c]["O"]
        for t in range(NT):
            g = 2 * t + par
            out[b, g * QTILE:(g + 1) * QTILE, :] = oc[t * QTILE:(t + 1) * QTILE, :]
    return out


# revision 15
# speedup vs baseline: 634333.0000x; 567370.0000x over previous
"""Causal self-attention (B=4, S=2048, D=1024, single head) on 8 trn2 cores.

v4: 128-row query slots with own-strips-first column permutation. Each core
takes natural query strips {2j+par} at permuted column block j; keys/V are in
the same permuted order (own strips at blocks 0-7, partner strips at 8-15).
Slot j attends permuted key blocks {0..j} u {8..8+j} (capacity 2j+2, total 72
vs 80 score/AV blocks for the 256-row tiling); host masks handle diagonal and
parity-dependent edge blocks. Scores run jointly per slot pair at N=256 over
the pair's shared kc list, plus two N=128 blocks for the odd slot.
"""

import numpy as np
from contextlib import ExitStack

import ml_dtypes

import concourse.bass as bass
import concourse.tile as tile
import concourse.mybir as mybir
from concourse import bacc
from concourse.bass_utils import run_bass_kernel_spmd

F32 = mybir.dt.float32
BF16 = mybir.dt.bfloat16
AFT = mybir.ActivationFunctionType
BF = ml_dtypes.bfloat16

B, S, D = 4, 2048, 1024
P = 128
QTILE = 256
NT = 4
DC = D // P
HK = S // P          # 16 key chunks of 128
SCALE = 1.0 / np.sqrt(D)
MASK_NEG = -1.0e9

_NC_CACHE = None


def _joint_list(u):
    """Shared kc list of slot pair (2u, 2u+1): {0..2u} u {8..8+2u}."""
    return list(range(0, 2 * u + 1)) + list(range(8, 8 + 2 * u + 1))


def _build():
    nc = bacc.Bacc("TRN2", target_bir_lowering=False, debug=False, num_devices=8)
    xt = nc.dram_tensor("XT", [D, S], BF16, kind="ExternalInput").ap()
    wqt = nc.dram_tensor("WqT", [D, D], BF16, kind="ExternalInput").ap()
    wkt = nc.dram_tensor("WkT", [D, D], BF16, kind="ExternalInput").ap()
    wvt = nc.dram_tensor("WvT", [D, D], BF16, kind="ExternalInput").ap()
    mskj = nc.dram_tensor("MaskJ", [4, 2, P, QTILE], F32, kind="ExternalInput").ap()
    mskb = nc.dram_tensor("MaskB", [4, 2, P, P], F32, kind="ExternalInput").ap()
    out = nc.dram_tensor("O", [8 * P, D], F32, kind="ExternalOutput").ap()

    with tile.TileContext(nc) as tc, ExitStack() as ctx:
        persist = ctx.enter_context(tc.tile_pool(name="persist", bufs=1))

        ones_f = persist.tile([P, 2], F32)
        nc.vector.memset(ones_f[:], 1.0)
        ones2 = persist.tile([P, 2], BF16)
        nc.vector.tensor_copy(ones2[:], ones_f[:])
        warm = persist.tile([P, 2], F32)
        nc.scalar.activation(warm[:], ones_f[:], AFT.Exp, scale=1.0)
        wz = persist.tile([P, 512], BF16)
        nc.vector.memset(wz[:], 0.0)

        KT = persist.tile([P, DC, S], BF16)
        V = persist.tile([P, HK, D], BF16)
        QT = persist.tile([P, DC, 8 * P], BF16)
        mtJ = persist.tile([P, 4, 2, QTILE], F32)
        mtB = persist.tile([P, 4, 2, P], F32)

        ev_ctr = [0]

        def evict(dst_ap, src_ap):
            ev_ctr[0] += 1
            if ev_ctr[0] % 2 == 0:
                nc.scalar.copy(dst_ap, src_ap)
            else:
                nc.vector.tensor_copy(dst_ap, src_ap)

        # ---------------- projections ----------------
        with tc.tile_pool(name="proj_in", bufs=1) as pin:
            XTs = pin.tile([P, DC, S], BF16, tag="xts")
            WkTs = pin.tile([P, DC, D], BF16, tag="wkts")
            WvTs = pin.tile([P, DC, D], BF16, tag="wvts")
            WqTs = pin.tile([P, DC, D], BF16, tag="wqts")
            for dc in range(DC):
                nc.sync.dma_start(XTs[:, dc, 0:S // 2], xt[dc * P:(dc + 1) * P, 0:S // 2])
                nc.sync.dma_start(WkTs[:, dc, :], wkt[dc * P:(dc + 1) * P, :])
            for dc in range(DC):
                nc.sync.dma_start(XTs[:, dc, S // 2:S], xt[dc * P:(dc + 1) * P, S // 2:S])
            for dc in range(DC):
                nc.gpsimd.dma_start(WqTs[:, dc, :], wqt[dc * P:(dc + 1) * P, :])
            for dc in range(DC):
                nc.sync.dma_start(WvTs[:, dc, :], wvt[dc * P:(dc + 1) * P, :])
            nc.scalar.dma_start(mtJ[:], mskj.rearrange("u w p j -> p u w j"))
            nc.scalar.dma_start(mtB[:], mskb.rearrange("u w p j -> p u w j"))

            with tc.tile_pool(name="proj_ps", bufs=8, space="PSUM") as kps:
                pwu = kps.tile([P, 512], F32, tag="pk", name="pwu")
                for i in range(9):
                    nc.tensor.matmul(pwu[:], wz[:, 0:P], wz[:],
                                     start=(i == 0), stop=(i == 8))
                for wave in range(4):
                    gset = [(4 * (wave // 2) + i, 2 * (wave % 2) + kch)
                            for i in range(4) for kch in range(2)]
                    pks = {g: kps.tile([P, 512], F32, tag="pk",
                                       name=f"pk_{g[0]}_{g[1]}") for g in gset}
                    for dc in range(DC):
                        for g in gset:
                            ec, kch = g
                            nc.tensor.matmul(pks[g][:], WkTs[:, dc, ec * P:(ec + 1) * P],
                                             XTs[:, dc, kch * 512:(kch + 1) * 512],
                                             start=(dc == 0), stop=(dc == DC - 1))
                    for g in gset:
                        ec, kch = g
                        evict(KT[:, ec, kch * 512:(kch + 1) * 512], pks[g][:])

                # Q^T: the core's 8 query slots are permuted columns 0..1023
                for ec in range(DC):
                    for qh in range(2):
                        pq = kps.tile([P, 512], F32, tag="pk", name=f"pq_{ec}_{qh}")
                        for dc in range(DC):
                            nc.tensor.matmul(pq[:], WqTs[:, dc, ec * P:(ec + 1) * P],
                                             XTs[:, dc, qh * 512:(qh + 1) * 512],
                                             start=(dc == 0), stop=(dc == DC - 1))
                        evict(QT[:, ec, qh * 512:(qh + 1) * 512], pq[:])
                for kb in range(HK):
                    for eh in range(2):
                        pv = kps.tile([P, 512], F32, tag="pk", name=f"pv_{kb}_{eh}")
                        for dc in range(DC):
                            nc.tensor.matmul(pv[:], XTs[:, dc, kb * P:(kb + 1) * P],
                                             WvTs[:, dc, eh * 512:(eh + 1) * 512],
                                             start=(dc == 0), stop=(dc == DC - 1))
                        evict(V[:, kb, eh * 512:(eh + 1) * 512], pv[:])

        # ---------------- attention ----------------
        with tc.tile_pool(name="attn", bufs=2) as pa, \
             tc.tile_pool(name="attn_e", bufs=2) as pe_pool, \
             tc.tile_pool(name="attn_o", bufs=2) as po, \
             tc.tile_pool(name="attn_s", bufs=3, space="PSUM") as psS, \
             tc.tile_pool(name="attn_u", bufs=2, space="PSUM") as psU, \
             tc.tile_pool(name="attn_r", bufs=1, space="PSUM") as psR:
            for u in range(4):
                jl = _joint_list(u)
                expS = pe_pool.tile([P, HK, QTILE], BF16, tag="expS")
                # joint scores for both slots of the pair, N=256
                for i, m in enumerate(jl):
                    pS = psS.tile([P, QTILE], F32, tag="pS")
                    for ec in range(DC):
                        nc.tensor.matmul(pS[:], KT[:, ec, m * P:(m + 1) * P],
                                         QT[:, ec, u * QTILE:(u + 1) * QTILE],
                                         start=(ec == 0), stop=(ec == DC - 1))
                    if m == 2 * u:
                        nc.vector.tensor_add(pS[:], pS[:], mtJ[:, u, 0, :])
                    elif m == 8 + 2 * u:
                        nc.vector.tensor_add(pS[:], pS[:], mtJ[:, u, 1, :])
                    nc.scalar.activation(expS[:, i, :], pS[:], AFT.Exp, scale=SCALE)
                # odd-slot-only blocks, N=128
                for w, m in enumerate((2 * u + 1, 9 + 2 * u)):
                    pSb = psS.tile([P, QTILE], F32, tag="pS")
                    for ec in range(DC):
                        nc.tensor.matmul(pSb[:, 0:P], KT[:, ec, m * P:(m + 1) * P],
                                         QT[:, ec, u * QTILE + P:(u + 1) * QTILE],
                                         start=(ec == 0), stop=(ec == DC - 1))
                    nc.vector.tensor_add(pSb[:, 0:P], pSb[:, 0:P], mtB[:, u, w, :])
                    nc.scalar.activation(expS[:, 14 + w, P:QTILE], pSb[:, 0:P],
                                         AFT.Exp, scale=SCALE)
                # AV per slot
                for sl in range(2):
                    idxs = list(range(len(jl)))
                    if sl == 1:
                        idxs += [14, 15]
                    col0 = sl * P
                    pU0 = psU.tile([P, 512], F32, tag="pU0")
                    pU1 = psU.tile([P, 512], F32, tag="pU1")
                    pR = psR.tile([P, 2], F32, tag="pR")
                    for ii, i in enumerate(idxs):
                        kc = jl[i] if i < len(jl) else (2 * u + 1 if i == 14 else 9 + 2 * u)
                        lhs = expS[:, i, col0:col0 + P]
                        st, sp = (ii == 0), (ii == len(idxs) - 1)
                        nc.tensor.matmul(pR[:], lhs, ones2[:], start=st, stop=sp)
                        nc.tensor.matmul(pU0[:], lhs, V[:, kc, 0:512], start=st, stop=sp)
                        nc.tensor.matmul(pU1[:], lhs, V[:, kc, 512:1024], start=st, stop=sp)
                    rsb = pa.tile([P, 1], F32, tag="rsb")
                    recip = pa.tile([P, 1], F32, tag="recip")
                    nc.vector.tensor_copy(rsb[:], pR[:, 0:1])
                    nc.vector.reciprocal(recip[:], rsb[:])
                    ot = po.tile([P, D], F32, tag="ot")
                    row0 = (2 * u + sl) * P
                    nc.vector.tensor_scalar_mul(ot[:, 0:512], pU0[:], recip[:])
                    nc.sync.dma_start(out[row0:row0 + P, 0:512], ot[:, 0:512])
                    nc.scalar.activation(ot[:, 512:1024], pU1[:], AFT.Copy,
                                         scale=recip[:])
                    nc.sync.dma_start(out[row0:row0 + P, 512:1024], ot[:, 512:1024])

    nc.compile()
    return nc


def _get_nc():
    global _NC_CACHE
    if _NC_CACHE is None:
        _NC_CACHE = _build()
    return _NC_CACHE


def _nat_strip(m, parity):
    """Natural 128-strip held at permuted block m (own-first layout)."""
    return 2 * m + parity if m < 8 else 2 * (m - 8) + 1 - parity


def _make_masks(parity):
    """MaskJ [4,2,128,256] for joint blocks {2u, 8+2u} (q = both slots);
    MaskB [4,2,128,128] for odd-slot blocks {2u+1, 9+2u}."""
    mj = np.empty((4, 2, P, QTILE), dtype=np.float32)
    mb = np.empty((4, 2, P, P), dtype=np.float32)
    ki = np.arange(P)[:, None]
    for u in range(4):
        s_a, s_b = 4 * u + parity, 4 * u + 2 + parity
        for w, m in enumerate((2 * u, 8 + 2 * u)):
            kglob = _nat_strip(m, parity) * P + ki
            qa = s_a * P + np.arange(P)[None, :]
            qb = s_b * P + np.arange(P)[None, :]
            mj[u, w, :, 0:P] = np.where(qa >= kglob, 0.0, MASK_NEG)
            mj[u, w, :, P:QTILE] = np.where(qb >= kglob, 0.0, MASK_NEG)
        for w, m in enumerate((2 * u + 1, 9 + 2 * u)):
            kglob = _nat_strip(m, parity) * P + ki
            qb = s_b * P + np.arange(P)[None, :]
            mb[u, w] = np.where(qb >= kglob, 0.0, MASK_NEG)
    return mj, mb


def kernel(X, W_q, W_k, W_v):
    X = np.asarray(X, dtype=np.float32)
    WqT = np.ascontiguousarray(np.asarray(W_q, dtype=np.float32).T).astype(BF)
    WkT = np.ascontiguousarray(np.asarray(W_k, dtype=np.float32).T).astype(BF)
    WvT = np.ascontiguousarray(np.asarray(W_v, dtype=np.float32).T).astype(BF)

    masks = [_make_masks(par) for par in range(2)]
    in_maps = []
    for c in range(8):
        b, par = c // 2, c % 2
        xt = np.asarray(X[b].T)  # [D, S] f32, natural
        cols = np.empty((D, S), dtype=np.float32)
        for m in range(16):
            s = _nat_strip(m, par)
            cols[:, m * P:(m + 1) * P] = xt[:, s * P:(s + 1) * P]
        in_maps.append({
            "XT": cols.astype(BF),
            "WqT": WqT, "WkT": WkT, "WvT": WvT,
            "MaskJ": masks[par][0], "MaskB": masks[par][1],
        })

    global _last_in_maps
    _last_in_maps = in_maps
    nc = _get_nc()
    res = run_bass_kernel_spmd(nc, in_maps, core_ids=list(range(8)))

    out = np.empty((B, S, D), dtype=np.float32)
    for c in range(8):
        b, par = c // 2, c % 2
        oc = res.results[c]["O"]
        for j in range(8):
            s = 2 * j + par
            out[b, s * P:(s + 1) * P, :] = oc[j * P:(j + 1) * P, :]
    return out
